# revision 2
# baseline (speedup 1.0000x reference)
"""Trainium2 Bass kernel v2 for nn_CombinatorialClassifier (segment_reduce).

Tensor-parallel over partitionings: core i owns partitionings {2i, 2i+1}
(a [2000, 2048] slice of W).

On-device pipeline per half h in {a, b}:
  logits_h = x @ W_h.T + b_h          (PE, fp16, 2 PSUM banks [64, 500])
  probs16_h = softmax(logits_h) fp16  (ACT exp + DVE normalize)
  pk_h [128, 1000, 4] fp16: pk[16g+r, k, l] = probs16_h[r + 16*l, k]
      built by 4 selection-mask matmuls (lhsT = const mask_l [64, 128])
      + 4 strided DVE copies PSUM -> SBUF. Every 16-row Q7 group holds
      all 64 batches, so the 8 Q7 cores each gather an independent
      1/8 class stream with d=4 (4 fp16 per index).
  4x ap_gather per half -> g_h_t [128, 1568, 4] fp16
Then per chunk t: DVE add g_a_t + g_b_t, DMA to out16 [128, 6272, 4] fp16.

Host: decode (row 16g+r, i, lane l) -> (batch r+16l, class 6250g+i),
sum the 8 per-core partials in fp32, normalize over classes, log.
"""

import os
from contextlib import ExitStack

import numpy as np

import concourse.bacc as bacc
import concourse.mybir as mybir
import concourse.tile as tile
from concourse import bass_utils

B, P, K, C, D = 64, 16, 1000, 50000, 2048
ESP = 1e-20
NCORES = 8
NLOC = 2 * K             # local logits width (2000)
NT = 500                 # matmul N-tile (1 PSUM bank)
DCH = D // 128           # 16 contraction chunks of 128
CG = C // 8              # classes per Q7 group stream (6250)
NCH = 4                  # gather chunks per half
JC = 1568                # classes per gather call per group (4*1568=6272>=6250)
CGP = NCH * JC           # padded group stream length (6272)
SCOL = JC // 16          # idx columns per call (98)

_F32 = mybir.dt.float32
_F16 = mybir.dt.float16
_I16 = mybir.dt.int16

_CACHE = {}
LAST_RESULTS = None


def _build_nc():
    nc = bacc.Bacc(
        "TRN2",
        target_bir_lowering=False,
        debug=False,
        enable_asserts=False,
        num_devices=NCORES,
    )
    xT_d = nc.dram_tensor("xT", [D, B], _F16, kind="ExternalInput")
    # W halves transposed: [2, D, 1000]; bias row per half: [2, 1000]
    wtb_d = nc.dram_tensor("wtb", [2, D, K], _F16, kind="ExternalInput")
    bias_d = nc.dram_tensor("bias", [2, K], _F16, kind="ExternalInput")
    idx_d = nc.dram_tensor("idx", [128, 2 * NCH * SCOL], _I16, kind="ExternalInput")
    mask_d = nc.dram_tensor("mask", [B, 4, 128], _F16, kind="ExternalInput")
    out_d = nc.dram_tensor("out16", [128, CGP, 4], _F16, kind="ExternalOutput")

    with tile.TileContext(nc) as tc, ExitStack() as ctx:
        const = ctx.enter_context(tc.tile_pool(name="const", bufs=1))
        wpool = ctx.enter_context(tc.tile_pool(name="w", bufs=3))
        ppool = ctx.enter_context(tc.tile_pool(name="p", bufs=2))
        kpool = ctx.enter_context(tc.tile_pool(name="k", bufs=2))
        spool = ctx.enter_context(tc.tile_pool(name="stats", bufs=2))
        gpool = ctx.enter_context(tc.tile_pool(name="g", bufs=8))
        psum = ctx.enter_context(tc.tile_pool(name="psum", bufs=4, space="PSUM"))
        psumk = ctx.enter_context(tc.tile_pool(name="psumk", bufs=4, space="PSUM"))

        xt = const.tile([128, DCH, B], _F16)
        nc.sync.dma_start(xt[:], xT_d.ap().rearrange("(c p) b -> p c b", p=128))
        ones = const.tile([1, B], _F16)
        nc.vector.memset(ones[:], 1.0)
        biast = const.tile([1, 2, K], _F16)
        nc.sync.dma_start(biast[:], bias_d.ap().rearrange("(o h) k -> o h k", o=1))
        idx_sb = const.tile([128, 2 * NCH * SCOL], _I16)
        nc.sync.dma_start(idx_sb[:], idx_d.ap())
        masks = const.tile([B, 4, 128], _F16)
        nc.sync.dma_start(masks[:], mask_d.ap())

        gtiles = [[None] * NCH for _ in range(2)]
        for h in range(2):
            # ---- W DMA: 4 x [128, 4, 1000] (1 MB each) ----
            wts = []
            for s in range(4):
                wt = wpool.tile([128, 4, K], _F16, tag="wt")
                nc.sync.dma_start(
                    wt[:],
                    wtb_d[h, 512 * s : 512 * (s + 1), :].rearrange(
                        "(q p) k -> p q k", p=128
                    ),
                )
                wts.append(wt)
            # ---- logits into 2 PSUM banks ----
            ps = [
                psum.tile([B, NT], _F32, tag="ps", name=f"ps{h}{n}")
                for n in range(2)
            ]
            for j in range(DCH):
                for n in range(2):
                    nc.tensor.matmul(
                        ps[n][:],
                        xt[:, j, :],
                        wts[j // 4][:, j % 4, NT * n : NT * (n + 1)],
                        start=(j == 0),
                        stop=False,
                    )
            for n in range(2):
                nc.tensor.matmul(
                    ps[n][:],
                    ones[:],
                    biast[:, h, NT * n : NT * (n + 1)],
                    start=False,
                    stop=True,
                )
            # ---- softmax -> probs16 [64, 1000] fp16 ----
            mx = spool.tile([B, 2], _F32, tag="mx")
            for n in range(2):
                nc.vector.reduce_max(
                    mx[:, n : n + 1], ps[n][:], axis=mybir.AxisListType.X
                )
            neg = spool.tile([B, 1], _F32, tag="neg")
            nc.vector.tensor_tensor(
                neg[:], mx[:, 0:1], mx[:, 1:2], op=mybir.AluOpType.max
            )
            nc.vector.tensor_scalar_mul(neg[:], neg[:], -1.0)
            sacc = spool.tile([B, 2], _F32, tag="sacc")
            probs = ppool.tile([B, K], _F16, tag="probs")
            for n in range(2):
                nc.scalar.activation(
                    probs[:, NT * n : NT * (n + 1)],
                    ps[n][:],
                    mybir.ActivationFunctionType.Exp,
                    bias=neg[:],
                    accum_out=sacc[:, n : n + 1],
                )
            rec = spool.tile([B, 1], _F32, tag="rec")
            nc.vector.tensor_tensor(
                rec[:], sacc[:, 0:1], sacc[:, 1:2], op=mybir.AluOpType.add
            )
            nc.vector.reciprocal(rec[:], rec[:])
            nc.vector.tensor_scalar_mul(probs[:], probs[:], rec[:])
            # ---- pack: pk[16g+r, k, l] = probs[r+16l, k] ----
            pk = kpool.tile([128, K, 4], _F16, tag="pk")
            for l in range(4):
                for n in range(2):
                    pp = psumk.tile([128, NT], _F32, tag="pp")
                    nc.tensor.matmul(
                        pp[:],
                        masks[:, l, :],
                        probs[:, NT * n : NT * (n + 1)],
                        start=True,
                        stop=True,
                    )
                    nc.vector.tensor_copy(pk[:, NT * n : NT * (n + 1), l], pp[:])
            # ---- gathers: 4 calls, one per class chunk ----
            for t in range(NCH):
                g = gpool.tile([128, JC, 4], _F16, tag="g")
                c0 = (NCH * h + t) * SCOL
                nc.gpsimd.ap_gather(
                    g[:],
                    pk[:],
                    idx_sb[:, c0 : c0 + SCOL],
                    channels=128,
                    num_elems=K,
                    d=4,
                    num_idxs=JC,
                )
                gtiles[h][t] = g

        # ---- combine halves + store ----
        for t in range(NCH):
            ga, gb = gtiles[0][t], gtiles[1][t]
            nc.vector.tensor_add(ga[:], ga[:], gb[:])
            nc.sync.dma_start(out_d[:, JC * t : JC * (t + 1), :], ga[:])

    nc.compile()
    return nc


def _host_inputs(x, W, b, part):
    """Per-core input maps: xT, wtb/bias halves, gather indices, masks."""
    xT = np.ascontiguousarray(x.T.astype(np.float16))
    part = np.asarray(part).astype(np.int64, copy=False)

    mask = np.zeros((B, 4, 128), np.float16)
    for l in range(4):
        for g in range(8):
            for r in range(16):
                mask[r + 16 * l, l, 16 * g + r] = 1.0

    in_maps = []
    for i in range(NCORES):
        wtb = np.empty((2, D, K), np.float16)
        bias = np.empty((2, K), np.float16)
        for h in range(2):
            r0 = NLOC * i + K * h
            wtb[h] = W[r0 : r0 + K].T
            bias[h] = b[r0 : r0 + K]

        # local partition ids in [0, K) per half
        loc = [part[2 * i + h] - (2 * i + h) * K for h in range(2)]
        idxh = np.zeros((128, 2 * NCH * SCOL), np.int16)
        for h in range(2):
            for t in range(NCH):
                cbase = 98 * (NCH * h + t)
                for g in range(8):
                    c0 = CG * g + JC * t
                    n = min(JC, max(0, CG - JC * t))
                    col = np.zeros(JC, np.int16)
                    col[:n] = loc[h][c0 : c0 + n]
                    blk = col.reshape(SCOL, 16).T  # elem j -> (row j%16, col j//16)
                    idxh[16 * g : 16 * (g + 1), cbase : cbase + SCOL] = blk
        in_maps.append(
            {"xT": xT, "wtb": wtb, "bias": bias, "idx": idxh, "mask": mask}
        )
    return in_maps


def _decode(arr):
    """[128, CGP, 4] fp16 partial -> [B, C] fp32."""
    a = np.asarray(arr, np.float32).reshape(8, 16, CGP, 4)  # [g, r, i, l]
    a = a.transpose(3, 1, 0, 2).reshape(B, 8, CGP)  # b = 16l + r
    return np.ascontiguousarray(a[:, :, :CG]).reshape(B, C)


def kernel(**inputs):
    global LAST_RESULTS
    x = np.asarray(inputs["input"], dtype=np.float32)
    W = np.asarray(inputs["W"], dtype=np.float32)
    b = np.asarray(inputs["b"], dtype=np.float32)
    part = np.asarray(inputs["partitionings"])
    assert x.shape == (B, D) and W.shape == (P * K, D)

    if "nc" not in _CACHE:
        _CACHE["nc"] = _build_nc()
    nc = _CACHE["nc"]

    in_maps = _host_inputs(x, W, b, part)
    trace = bool(int(os.environ.get("BASSK_TRACE", "0")))
    res = bass_utils.run_bass_kernel_spmd(
        nc,
        in_maps,
        core_ids=list(range(NCORES)),
        trace=trace,
        tmpdir=os.environ.get("BASSK_TRACE_DIR") or None,
    )
    LAST_RESULTS = res

    acc = _decode(res.results[0]["out16"])
    for i in range(1, NCORES):
        acc += _decode(res.results[i]["out16"])
    tot = acc.sum(axis=1, keepdims=True)
    return np.log(acc / tot + ESP).astype(np.float32)


# revision 3
# speedup vs baseline: 1.0065x; 1.0065x over previous
"""Trainium2 Bass kernel v2 for nn_CombinatorialClassifier (segment_reduce).

Tensor-parallel over partitionings: core i owns partitionings {2i, 2i+1}
(a [2000, 2048] slice of W).

On-device pipeline per half h in {a, b}:
  logits_h = x @ W_h.T + b_h          (PE, fp16, 2 PSUM banks [64, 500])
  probs16_h = softmax(logits_h) fp16  (ACT exp + DVE normalize)
  pk_h [128, 1000, 4] fp16: pk[16g+r, k, l] = probs16_h[r + 16*l, k]
      built by 4 selection-mask matmuls (lhsT = const mask_l [64, 128])
      + 4 strided DVE copies PSUM -> SBUF. Every 16-row Q7 group holds
      all 64 batches, so the 8 Q7 cores each gather an independent
      1/8 class stream with d=4 (4 fp16 per index).
  4x ap_gather per half -> g_h_t [128, 1568, 4] fp16
Then per chunk t: DVE add g_a_t + g_b_t, DMA to out16 [128, 6272, 4] fp16.

Host: decode (row 16g+r, i, lane l) -> (batch r+16l, class 6250g+i),
sum the 8 per-core partials in fp32, normalize over classes, log.
"""

import os
from contextlib import ExitStack

import numpy as np

import concourse.bacc as bacc
import concourse.mybir as mybir
import concourse.tile as tile
from concourse import bass_utils

B, P, K, C, D = 64, 16, 1000, 50000, 2048
ESP = 1e-20
NCORES = 8
NLOC = 2 * K             # local logits width (2000)
NT = 500                 # matmul N-tile (1 PSUM bank)
DCH = D // 128           # 16 contraction chunks of 128
CG = C // 8              # classes per Q7 group stream (6250)
NCH = 4                  # gather chunks per half
JC = 1568                # classes per gather call per group (4*1568=6272>=6250)
CGP = NCH * JC           # padded group stream length (6272)
SCOL = JC // 16          # idx columns per call (98)

_F32 = mybir.dt.float32
_F16 = mybir.dt.float16
_I16 = mybir.dt.int16

_CACHE = {}
LAST_RESULTS = None


def _build_nc():
    nc = bacc.Bacc(
        "TRN2",
        target_bir_lowering=False,
        debug=False,
        enable_asserts=False,
        num_devices=NCORES,
    )
    # pre-swizzled on host: xT[p, c, b] = x[b, 128c + p]
    xT_d = nc.dram_tensor("xT", [128, DCH, B], _F16, kind="ExternalInput")
    # pre-swizzled W: wtb[h, s, p, q, k] = W[2000i + 1000h + k, 512s + 128q + p]
    wtb_d = nc.dram_tensor("wtb", [2, 4, 128, 4, K], _F16, kind="ExternalInput")
    bias_d = nc.dram_tensor("bias", [2, K], _F16, kind="ExternalInput")
    idx_d = nc.dram_tensor("idx", [128, 2 * NCH * SCOL], _I16, kind="ExternalInput")
    mask_d = nc.dram_tensor("mask", [B, 4, 128], _F16, kind="ExternalInput")
    out_d = nc.dram_tensor("out16", [128, CGP, 4], _F16, kind="ExternalOutput")

    with tile.TileContext(nc) as tc, ExitStack() as ctx:
        const = ctx.enter_context(tc.tile_pool(name="const", bufs=1))
        wpool = ctx.enter_context(tc.tile_pool(name="w", bufs=3))
        ppool = ctx.enter_context(tc.tile_pool(name="p", bufs=2))
        kpool = ctx.enter_context(tc.tile_pool(name="k", bufs=2))
        spool = ctx.enter_context(tc.tile_pool(name="stats", bufs=2))
        gpool = ctx.enter_context(tc.tile_pool(name="g", bufs=8))
        psum = ctx.enter_context(tc.tile_pool(name="psum", bufs=4, space="PSUM"))
        psumk = ctx.enter_context(tc.tile_pool(name="psumk", bufs=4, space="PSUM"))

        xt = const.tile([128, DCH, B], _F16)
        nc.sync.dma_start(xt[:], xT_d.ap())
        ones = const.tile([1, B], _F16)
        nc.vector.memset(ones[:], 1.0)
        biast = const.tile([1, 2, K], _F16)
        nc.sync.dma_start(biast[:], bias_d.ap().rearrange("(o h) k -> o h k", o=1))
        idx_sb = const.tile([128, 2 * NCH * SCOL], _I16)
        nc.sync.dma_start(idx_sb[:], idx_d.ap())
        masks = const.tile([B, 4, 128], _F16)
        nc.sync.dma_start(masks[:], mask_d.ap())

        gtiles = [[None] * NCH for _ in range(2)]
        for h in range(2):
            # ---- W DMA: 4 x [128, 4, 1000] (1 MB each) ----
            wts = []
            for s in range(4):
                wt = wpool.tile([128, 4, K], _F16, tag="wt")
                nc.sync.dma_start(wt[:], wtb_d[h, s])
                wts.append(wt)
            # ---- logits into 2 PSUM banks ----
            ps = [
                psum.tile([B, NT], _F32, tag="ps", name=f"ps{h}{n}")
                for n in range(2)
            ]
            for j in range(DCH):
                for n in range(2):
                    nc.tensor.matmul(
                        ps[n][:],
                        xt[:, j, :],
                        wts[j // 4][:, j % 4, NT * n : NT * (n + 1)],
                        start=(j == 0),
                        stop=False,
                    )
            for n in range(2):
                nc.tensor.matmul(
                    ps[n][:],
                    ones[:],
                    biast[:, h, NT * n : NT * (n + 1)],
                    start=False,
                    stop=True,
                )
            # ---- softmax -> probs16 [64, 1000] fp16 ----
            mx = spool.tile([B, 2], _F32, tag="mx")
            for n in range(2):
                nc.vector.reduce_max(
                    mx[:, n : n + 1], ps[n][:], axis=mybir.AxisListType.X
                )
            neg = spool.tile([B, 1], _F32, tag="neg")
            nc.vector.tensor_tensor(
                neg[:], mx[:, 0:1], mx[:, 1:2], op=mybir.AluOpType.max
            )
            nc.vector.tensor_scalar_mul(neg[:], neg[:], -1.0)
            sacc = spool.tile([B, 2], _F32, tag="sacc")
            probs = ppool.tile([B, K], _F16, tag="probs")
            for n in range(2):
                nc.scalar.activation(
                    probs[:, NT * n : NT * (n + 1)],
                    ps[n][:],
                    mybir.ActivationFunctionType.Exp,
                    bias=neg[:],
                    accum_out=sacc[:, n : n + 1],
                )
            rec = spool.tile([B, 1], _F32, tag="rec")
            nc.vector.tensor_tensor(
                rec[:], sacc[:, 0:1], sacc[:, 1:2], op=mybir.AluOpType.add
            )
            nc.vector.reciprocal(rec[:], rec[:])
            nc.vector.tensor_scalar_mul(probs[:], probs[:], rec[:])
            # ---- pack: pk[16g+r, k, l] = probs[r+16l, k] ----
            pk = kpool.tile([128, K, 4], _F16, tag="pk")
            for l in range(4):
                for n in range(2):
                    pp = psumk.tile([128, NT], _F32, tag="pp")
                    nc.tensor.matmul(
                        pp[:],
                        masks[:, l, :],
                        probs[:, NT * n : NT * (n + 1)],
                        start=True,
                        stop=True,
                    )
                    if l < 2:
                        nc.vector.tensor_copy(pk[:, NT * n : NT * (n + 1), l], pp[:])
                    else:
                        nc.scalar.copy(pk[:, NT * n : NT * (n + 1), l], pp[:])
            # ---- gathers: 4 calls, one per class chunk ----
            for t in range(NCH):
                g = gpool.tile([128, JC, 4], _F16, tag="g")
                c0 = (NCH * h + t) * SCOL
                nc.gpsimd.ap_gather(
                    g[:],
                    pk[:],
                    idx_sb[:, c0 : c0 + SCOL],
                    channels=128,
                    num_elems=K,
                    d=4,
                    num_idxs=JC,
                )
                gtiles[h][t] = g

        # ---- combine halves + store ----
        for t in range(NCH):
            ga, gb = gtiles[0][t], gtiles[1][t]
            nc.vector.tensor_add(ga[:], ga[:], gb[:])
            nc.scalar.dma_start(out_d[:, JC * t : JC * (t + 1), :], ga[:])

    nc.compile()
    return nc


def _host_inputs(x, W, b, part):
    """Per-core input maps: xT, wtb/bias halves, gather indices, masks."""
    # xT[p, c, b] = x[b, 128c + p]
    xT = np.ascontiguousarray(
        x.T.astype(np.float16).reshape(DCH, 128, B).transpose(1, 0, 2)
    )
    part = np.asarray(part).astype(np.int64, copy=False)

    mask = np.zeros((B, 4, 128), np.float16)
    for l in range(4):
        for g in range(8):
            for r in range(16):
                mask[r + 16 * l, l, 16 * g + r] = 1.0

    in_maps = []
    for i in range(NCORES):
        # wtb[h, s, p, q, k] = W[2000i + 1000h + k, 512s + 128q + p]
        wtb = np.empty((2, 4, 128, 4, K), np.float16)
        bias = np.empty((2, K), np.float16)
        for h in range(2):
            r0 = NLOC * i + K * h
            wh = W[r0 : r0 + K].T.astype(np.float16)  # [col, k]
            wtb[h] = wh.reshape(4, 4, 128, K).transpose(0, 2, 1, 3)
            bias[h] = b[r0 : r0 + K]

        # local partition ids in [0, K) per half
        loc = [part[2 * i + h] - (2 * i + h) * K for h in range(2)]
        idxh = np.zeros((128, 2 * NCH * SCOL), np.int16)
        for h in range(2):
            for t in range(NCH):
                cbase = 98 * (NCH * h + t)
                for g in range(8):
                    c0 = CG * g + JC * t
                    n = min(JC, max(0, CG - JC * t))
                    col = np.zeros(JC, np.int16)
                    col[:n] = loc[h][c0 : c0 + n]
                    blk = col.reshape(SCOL, 16).T  # elem j -> (row j%16, col j//16)
                    idxh[16 * g : 16 * (g + 1), cbase : cbase + SCOL] = blk
        in_maps.append(
            {"xT": xT, "wtb": wtb, "bias": bias, "idx": idxh, "mask": mask}
        )
    return in_maps


def _decode(arr):
    """[128, CGP, 4] fp16 partial -> [B, C] fp32."""
    a = np.asarray(arr, np.float32).reshape(8, 16, CGP, 4)  # [g, r, i, l]
    a = a.transpose(3, 1, 0, 2).reshape(B, 8, CGP)  # b = 16l + r
    return np.ascontiguousarray(a[:, :, :CG]).reshape(B, C)


def kernel(**inputs):
    global LAST_RESULTS
    x = np.asarray(inputs["input"], dtype=np.float32)
    W = np.asarray(inputs["W"], dtype=np.float32)
    b = np.asarray(inputs["b"], dtype=np.float32)
    part = np.asarray(inputs["partitionings"])
    assert x.shape == (B, D) and W.shape == (P * K, D)

    if "nc" not in _CACHE:
        _CACHE["nc"] = _build_nc()
    nc = _CACHE["nc"]

    in_maps = _host_inputs(x, W, b, part)
    trace = bool(int(os.environ.get("BASSK_TRACE", "0")))
    res = bass_utils.run_bass_kernel_spmd(
        nc,
        in_maps,
        core_ids=list(range(NCORES)),
        trace=trace,
        tmpdir=os.environ.get("BASSK_TRACE_DIR") or None,
    )
    LAST_RESULTS = res

    acc = _decode(res.results[0]["out16"])
    for i in range(1, NCORES):
        acc += _decode(res.results[i]["out16"])
    tot = acc.sum(axis=1, keepdims=True)
    return np.log(acc / tot + ESP).astype(np.float32)
